# revision 1
# baseline (speedup 1.0000x reference)
"""GQA with RoPE + sliding-window causal attention on 8 TRN2 NeuronCores.

Sharding: batch (2) x KV-groups (4) -> 8 cores, pure SPMD (no collectives).
Each core computes q/k/v projections for its (batch, group), RoPE, windowed
attention (window=512), and a partial output projection against its group's
WO columns. Host sums the 4 group partials per batch element.

Layout:
  * Weights pre-permuted on host so each head's 64 dims are deinterleaved
    ([even | odd]) -> RoPE is two contiguous 32-wide halves addressed with
    one strided free-dim AP across all 4 q heads + k per DVE op.
  * q/k are transposed on PE (identity matmul) into a unified dims-major
    tensor qkT[64, 5, T] (4 q heads + k); one fused ACT copy drains the
    5 transposes per row tile.
  * Scores are computed keys-on-partitions with kT stationary and all 4
    heads batched in the moving free dim (N=512, full fp32r rate), one
    matmul per 128-key block into a 5-bank PSUM tile; a single Exp
    (scale=1/8, no max-subtraction; scores ~ N(0,1)) emits bf16 probs;
    sliding-window masking is a binary multiply on the two boundary key
    blocks (GPSIMD, off the critical DVE path).
  * AV runs probsT-stationary against [v | 1] so column 64 accumulates the
    softmax denominator; normalization is a per-partition DVE scale.
  * attn is transposed back on PE and immediately consumed by the WO
    partial projection, so output DMA is spread across the whole kernel.
"""

import sys

sys.path.insert(0, "/opt/trn_rl_repo")

import numpy as np
from contextlib import ExitStack

D_MODEL = 1024
GROUP_SIZE = 4
NUM_GROUPS = 4
D_K = 64
THETA = 10000.0
WINDOW = 512
T = 2048
B = 2
NT = T // 128  # 16 row tiles
HALF = D_K // 2

_PROGRAM = None


def _build_program():
    from concourse import bacc, tile
    import concourse.mybir as mybir

    f32 = mybir.dt.float32
    f32r = mybir.dt.float32r
    bf16 = mybir.dt.bfloat16
    Exp = mybir.ActivationFunctionType.Exp
    mult = mybir.AluOpType.mult
    subtract = mybir.AluOpType.subtract
    add = mybir.AluOpType.add

    nc = bacc.Bacc("TRN2", target_bir_lowering=False, debug=False, num_devices=8)

    xt_d = nc.dram_tensor("xt", [NT, 128, 1024], f32r, kind="ExternalInput").ap()
    wq_d = nc.dram_tensor("wqkvT", [128, 8, 384], f32r, kind="ExternalInput").ap()
    wo_d = nc.dram_tensor("woT", [128, 2, 1024], bf16, kind="ExternalInput").ap()
    cos_d = nc.dram_tensor("cos5", [128, NT, 5, HALF], f32, kind="ExternalInput").ap()
    sin_d = nc.dram_tensor("sin5", [128, NT, 5, HALF], f32, kind="ExternalInput").ap()
    md_d = nc.dram_tensor("maskd", [128, 256], bf16, kind="ExternalInput").ap()
    mo_d = nc.dram_tensor("masko", [128, 256], bf16, kind="ExternalInput").ap()
    id_d = nc.dram_tensor("ident", [128, 128], f32, kind="ExternalInput").ap()
    id16_d = nc.dram_tensor("ident16", [128, 128], bf16, kind="ExternalInput").ap()
    out_d = nc.dram_tensor("out", [T, D_MODEL], f32, kind="ExternalOutput").ap()

    with tile.TileContext(nc) as tc:
        with ExitStack() as ctx:
            persist = ctx.enter_context(tc.tile_pool(name="persist", bufs=1))
            wq_sb = persist.tile([128, 8, 384], f32r, tag="wq")
            wo_sb = persist.tile([128, 2, 1024], bf16, tag="wo")
            cos_sb = persist.tile([128, NT, 5, HALF], f32, tag="cos")
            sin_sb = persist.tile([128, NT, 5, HALF], f32, tag="sin")
            md_sb = persist.tile([128, 256], bf16, tag="md")
            mo_sb = persist.tile([128, 256], bf16, tag="mo")
            id_sb = persist.tile([128, 128], f32, tag="id")
            id16_sb = persist.tile([128, 128], bf16, tag="id16")
            qk_sb = persist.tile([64, 5, T], bf16, tag="qk")  # dims-major q(4)+k
            v_sb = persist.tile([128, NT, 65], bf16, tag="v")  # [v | 1] per key block

            nc.sync.dma_start(wq_sb[:], wq_d[:])
            nc.sync.dma_start(wo_sb[:], wo_d[:])
            nc.sync.dma_start(cos_sb[:], cos_d[:])
            nc.sync.dma_start(sin_sb[:], sin_d[:])
            nc.sync.dma_start(md_sb[:], md_d[:])
            nc.sync.dma_start(mo_sb[:], mo_d[:])
            nc.sync.dma_start(id_sb[:], id_d[:])
            nc.sync.dma_start(id16_sb[:], id16_d[:])

            # ---------------- phase 1: QKV projection + RoPE + transposes
            with ExitStack() as c1:
                xt_pool = c1.enter_context(tc.tile_pool(name="xtp", bufs=2))
                rot_pool = c1.enter_context(tc.tile_pool(name="rotp", bufs=2))
                tmp_pool = c1.enter_context(tc.tile_pool(name="tmpp", bufs=2))
                pp_pool = c1.enter_context(
                    tc.tile_pool(name="ppp", bufs=2, space="PSUM")
                )
                ptr_pool = c1.enter_context(
                    tc.tile_pool(name="ptrp", bufs=2, space="PSUM")
                )
                for tt in range(NT):
                    xt = xt_pool.tile([128, 8, 128], f32r, tag="xt")
                    nc.sync.dma_start(xt[:], xt_d[tt])
                    pp = pp_pool.tile([128, 6, 64], f32, tag="pp")
                    for kt in range(8):
                        nc.tensor.matmul(
                            pp[:],
                            lhsT=xt[:, kt, :],
                            rhs=wq_sb[:, kt, :],
                            start=(kt == 0),
                            stop=(kt == 7),
                        )
                    a = pp[:, 0:5, 0:HALF]
                    b = pp[:, 0:5, HALF:D_K]
                    co = cos_sb[:, tt, :, :]
                    si = sin_sb[:, tt, :, :]
                    rot = rot_pool.tile([128, 5, 64], bf16, tag="rot")
                    t1 = tmp_pool.tile([128, 5, HALF], f32, tag="t1")
                    t2 = tmp_pool.tile([128, 5, HALF], f32, tag="t2")
                    nc.vector.tensor_tensor(t1[:], a, co, mult)
                    nc.vector.tensor_tensor(t2[:], b, si, mult)
                    nc.gpsimd.tensor_tensor(rot[:, :, 0:HALF], t1[:], t2[:], subtract)
                    t3 = tmp_pool.tile([128, 5, HALF], f32, tag="t1")
                    t4 = tmp_pool.tile([128, 5, HALF], f32, tag="t2")
                    nc.vector.tensor_tensor(t3[:], a, si, mult)
                    nc.vector.tensor_tensor(t4[:], b, co, mult)
                    nc.gpsimd.tensor_tensor(rot[:, :, HALF:D_K], t3[:], t4[:], add)
                    nc.vector.tensor_copy(v_sb[:, tt, 0:64], pp[:, 5, :])
                    nc.vector.memset(v_sb[:, tt, 64:65], 1.0)
                    pt = ptr_pool.tile([64, 5, 128], bf16, tag="pt")
                    for hh in range(5):
                        nc.tensor.transpose(pt[:, hh, :], rot[:, hh, :], id16_sb[:])
                    nc.scalar.copy(qk_sb[:, :, tt * 128 : (tt + 1) * 128], pt[:])

            # ---------------- phase 2+3: attention + WO partial projection
            with ExitStack() as c2:
                sc_pool = c2.enter_context(
                    tc.tile_pool(name="scp", bufs=2, space="PSUM")
                )
                mix_pool = c2.enter_context(
                    tc.tile_pool(name="mixp", bufs=2, space="PSUM")
                )
                pr_pool = c2.enter_context(tc.tile_pool(name="prp", bufs=2))
                pre_pool = c2.enter_context(tc.tile_pool(name="prep", bufs=4))
                attn_pool = c2.enter_context(tc.tile_pool(name="attnp", bufs=2))
                at_pool = c2.enter_context(tc.tile_pool(name="atp", bufs=2))
                rc_pool = c2.enter_context(tc.tile_pool(name="rcp", bufs=2))
                ob_pool = c2.enter_context(tc.tile_pool(name="obp", bufs=2))
                for i in range(NT):
                    kb0 = max(0, i - 4)
                    nkb = i - kb0 + 1
                    edge_old = i >= 4
                    prs = []
                    pre_d = []
                    pre_o = []
                    for hp in range(2):
                        sc = sc_pool.tile([128, 5, 256], f32, tag="sc")
                        for j in range(nkb):
                            kb = kb0 + j
                            nc.tensor.matmul(
                                sc[:, j, :],
                                lhsT=qk_sb[:, 4, kb * 128 : (kb + 1) * 128],
                                rhs=qk_sb[
                                    :, hp * 2 : hp * 2 + 2, i * 128 : (i + 1) * 128
                                ],
                                start=True,
                                stop=True,
                            )
                        pr = pr_pool.tile([128, 5, 256], bf16, tag="pr")
                        nc.scalar.activation(
                            pr[:, 0:nkb, :], sc[:, 0:nkb, :], Exp, scale=0.125
                        )
                        ed = pre_pool.tile([128, 256], bf16, tag="ed")
                        nc.vector.tensor_tensor(
                            ed[:], pr[:, nkb - 1, :], md_sb[:], mult
                        )
                        eo = None
                        if edge_old:
                            eo = pre_pool.tile([128, 256], bf16, tag="eo")
                            nc.vector.tensor_tensor(
                                eo[:], pr[:, 0, :], mo_sb[:], mult
                            )
                        prs.append(pr)
                        pre_d.append(ed)
                        pre_o.append(eo)
                    av = mix_pool.tile([128, 4, 65], f32, tag="m")
                    unmasked = [
                        j for j in range(nkb - 1) if not (j == 0 and edge_old)
                    ]
                    masked = ([0] if edge_old else []) + [nkb - 1]
                    order = unmasked + masked
                    for h in range(4):
                        hp, hq = h // 2, h % 2
                        for pos, j in enumerate(order):
                            kb = kb0 + j
                            if j == nkb - 1:
                                lhs = pre_d[hp][:, hq * 128 : (hq + 1) * 128]
                            elif j == 0 and edge_old:
                                lhs = pre_o[hp][:, hq * 128 : (hq + 1) * 128]
                            else:
                                lhs = prs[hp][:, j, hq * 128 : (hq + 1) * 128]
                            nc.tensor.matmul(
                                av[:, h, :],
                                lhsT=lhs,
                                rhs=v_sb[:, kb, :],
                                start=(pos == 0),
                                stop=(pos == len(order) - 1),
                            )
                    rc = rc_pool.tile([128, 4, 1], f32, tag="rc")
                    nc.vector.reciprocal(rc[:], av[:, :, 64:65])
                    attn = attn_pool.tile([128, 4, 64], bf16, tag="attn")
                    nc.vector.tensor_tensor(
                        attn[:],
                        av[:, :, 0:64],
                        rc[:, :, 0:1].broadcast_to((128, 4, 64)),
                        mult,
                    )
                    at = at_pool.tile([128, 2, 128], bf16, tag="at")
                    atp = mix_pool.tile([128, 2, 128], bf16, tag="m")
                    for xx in range(2):
                        nc.tensor.transpose(
                            atp[:, xx, :],
                            attn[:, xx * 2 : (xx + 1) * 2, :],
                            id16_sb[:],
                        )
                    nc.vector.tensor_copy(at[:], atp[:])
                    # WO partial projection for this row tile
                    for nb in range(2):
                        po = mix_pool.tile([128, 512], f32, tag="m")
                        for kb2 in range(2):
                            nc.tensor.matmul(
                                po[:],
                                lhsT=at[:, kb2, :],
                                rhs=wo_sb[:, kb2, nb * 512 : (nb + 1) * 512],
                                start=(kb2 == 0),
                                stop=(kb2 == 1),
                            )
                        ob = ob_pool.tile([128, 512], f32, tag="ob")
                        if nb == 0:
                            nc.scalar.copy(ob[:], po[:])
                        else:
                            nc.vector.tensor_copy(ob[:], po[:])
                        nc.sync.dma_start(
                            out_d[i * 128 : (i + 1) * 128, nb * 512 : (nb + 1) * 512],
                            ob[:],
                        )

    nc.compile()
    return nc


def _host_inputs(x, WQ, WK, WV, WO, token_positions):
    import ml_dtypes

    perm64 = np.concatenate([np.arange(0, 64, 2), np.arange(1, 64, 2)])
    pos = np.asarray(token_positions).astype(np.float64)
    inv_freq = THETA ** (-np.arange(HALF, dtype=np.float64) / HALF)
    ang = pos[:, None] * inv_freq[None, :]
    cosr = np.cos(ang).astype(np.float32)
    sinr = np.sin(ang).astype(np.float32)

    def _rope_tiles(r):
        c = r.reshape(NT, 128, HALF).transpose(1, 0, 2)
        return np.ascontiguousarray(
            np.broadcast_to(c[:, :, None, :], (128, NT, 5, HALF))
        )

    cos5 = _rope_tiles(cosr)
    sin5 = _rope_tiles(sinr)

    rk = np.arange(128)[:, None]
    r = np.arange(128)[None, :]
    maskd = np.tile((rk <= r).astype(np.float32), (1, 2)).astype(ml_dtypes.bfloat16)
    masko = np.tile((rk >= r).astype(np.float32), (1, 2)).astype(ml_dtypes.bfloat16)
    ident = np.eye(128, dtype=np.float32)
    ident16 = np.eye(128).astype(ml_dtypes.bfloat16)

    in_maps = []
    for core in range(8):
        bi, g = core // 4, core % 4
        WQp = (
            WQ[g * 256 : (g + 1) * 256]
            .reshape(GROUP_SIZE, D_K, D_MODEL)[:, perm64, :]
            .reshape(256, D_MODEL)
        )
        WKp = WK[g * 64 : (g + 1) * 64][perm64, :]
        Wf = np.concatenate([WQp, WKp, WV[g * 64 : (g + 1) * 64]], axis=0)
        wqkvT = np.ascontiguousarray(Wf.T.reshape(8, 128, 384).transpose(1, 0, 2))
        woT = np.ascontiguousarray(
            WO[:, g * 256 : (g + 1) * 256].T.reshape(2, 128, 1024).transpose(1, 0, 2)
        ).astype(ml_dtypes.bfloat16)
        xT = np.ascontiguousarray(x[bi].T)
        xt4 = np.ascontiguousarray(
            xT.reshape(8, 128, NT, 128).transpose(2, 1, 0, 3).reshape(NT, 128, 1024)
        )
        in_maps.append(
            {
                "xt": xt4,
                "wqkvT": wqkvT,
                "woT": woT,
                "cos5": cos5,
                "sin5": sin5,
                "maskd": maskd,
                "masko": masko,
                "ident": ident,
                "ident16": ident16,
            }
        )
    return in_maps


def kernel(x, WQ, WK, WV, WO, token_positions):
    global _PROGRAM
    from concourse.bass_utils import run_bass_kernel_spmd

    x = np.asarray(x, dtype=np.float32)
    WQ = np.asarray(WQ, dtype=np.float32)
    WK = np.asarray(WK, dtype=np.float32)
    WV = np.asarray(WV, dtype=np.float32)
    WO = np.asarray(WO, dtype=np.float32)

    if _PROGRAM is None:
        _PROGRAM = _build_program()
    nc = _PROGRAM

    in_maps = _host_inputs(x, WQ, WK, WV, WO, token_positions)
    res = run_bass_kernel_spmd(nc, in_maps, core_ids=list(range(8)))
    out = np.zeros((B, T, D_MODEL), dtype=np.float32)
    for core in range(8):
        out[core // 4] += res.results[core]["out"]
    return out



# revision 4
# speedup vs baseline: 1.1661x; 1.1661x over previous
"""GQA with RoPE + sliding-window causal attention on 8 TRN2 NeuronCores.

Sharding: batch (2) x KV-groups (4) -> 8 cores, pure SPMD (no collectives).
Each core computes q/k/v projections for its (batch, group), RoPE, windowed
attention (window=512), and a partial output projection against its group's
WO columns. Host sums the 4 group partials per batch element.

v2 layout/perf notes:
  * All 2-byte tensors are fp16 (same PE/DVE rate as bf16, 8x the mantissa):
    x tiles, W_qkv, W_o, qk_sb, v_sb, masks, probs, attn.
  * Weights pre-permuted on host so each head's 64 dims are deinterleaved
    ([even | odd]) -> RoPE is two contiguous 32-wide halves per DVE op.
  * cos/sin stored deduplicated [128, NT, 1, 32] f32 and broadcast across
    the 5 heads with a stride-0 AP dim.
  * Both phases are software-pipelined so the PE stream stays dense:
    phase 1 issues proj(i) then transpose/drain(i-1); phase 2 issues
    scores/exp/mask(i) then AV..WO(i-1).
  * WO partials drain PSUM->SBUF as fp16 into one [128,1024] buffer
    (Pool engine), one output DMA per row tile; host sums fp16 partials.
"""

import sys

sys.path.insert(0, "/opt/trn_rl_repo")

import numpy as np
from contextlib import ExitStack

D_MODEL = 1024
GROUP_SIZE = 4
NUM_GROUPS = 4
D_K = 64
THETA = 10000.0
WINDOW = 512
T = 2048
B = 2
NT = T // 128  # 16 row tiles
HALF = D_K // 2

_PROGRAM = None


def _build_program():
    from concourse import bacc, tile
    import concourse.mybir as mybir

    f32 = mybir.dt.float32
    f16 = mybir.dt.float16
    Exp = mybir.ActivationFunctionType.Exp
    mult = mybir.AluOpType.mult
    subtract = mybir.AluOpType.subtract
    add = mybir.AluOpType.add

    nc = bacc.Bacc("TRN2", target_bir_lowering=False, debug=False, num_devices=8)

    xt_d = nc.dram_tensor("xt", [NT // 2, 128, 2, 8, 128], f16, kind="ExternalInput").ap()
    wq_d = nc.dram_tensor("wqkvT", [128, 8, 384], f16, kind="ExternalInput").ap()
    wo_d = nc.dram_tensor("woT", [128, 2, 1024], f16, kind="ExternalInput").ap()
    cos_d = nc.dram_tensor("cosb", [128, NT, 1, HALF], f32, kind="ExternalInput").ap()
    sin_d = nc.dram_tensor("sinb", [128, NT, 1, HALF], f32, kind="ExternalInput").ap()
    md_d = nc.dram_tensor("maskd", [128, 256], f16, kind="ExternalInput").ap()
    mo_d = nc.dram_tensor("masko", [128, 256], f16, kind="ExternalInput").ap()
    id16_d = nc.dram_tensor("ident16", [128, 128], f16, kind="ExternalInput").ap()
    out_d = nc.dram_tensor("out", [T, D_MODEL], f16, kind="ExternalOutput").ap()

    with tile.TileContext(nc) as tc:
        with ExitStack() as ctx:
            persist = ctx.enter_context(tc.tile_pool(name="persist", bufs=1))
            wq_sb = persist.tile([128, 8, 384], f16, tag="wq")
            wo_sb = persist.tile([128, 2, 1024], f16, tag="wo")
            cos_sb = persist.tile([128, NT, 1, HALF], f32, tag="cos")
            sin_sb = persist.tile([128, NT, 1, HALF], f32, tag="sin")
            md_sb = persist.tile([128, 256], f16, tag="md")
            mo_sb = persist.tile([128, 256], f16, tag="mo")
            id16_sb = persist.tile([128, 128], f16, tag="id16")
            qk_sb = persist.tile([64, 5, T], f16, tag="qk")  # dims-major q(4)+k
            v_sb = persist.tile([128, NT, 65], f16, tag="v")  # [v | 1] per key block

            # preloads needed by phase 1 first; wo/masks late (phase 2 use)
            nc.sync.dma_start(wq_sb[:], wq_d[:])
            nc.sync.dma_start(id16_sb[:], id16_d[:])
            nc.sync.dma_start(cos_sb[:], cos_d[:])
            nc.sync.dma_start(sin_sb[:], sin_d[:])

            # ---------------- phase 1: QKV projection + RoPE + transposes
            with ExitStack() as c1:
                xt_pool = c1.enter_context(tc.tile_pool(name="xtp", bufs=2))
                rot_pool = c1.enter_context(tc.tile_pool(name="rotp", bufs=2))
                tmp_pool = c1.enter_context(tc.tile_pool(name="tmpp", bufs=2))
                pp_pool = c1.enter_context(
                    tc.tile_pool(name="ppp", bufs=2, space="PSUM")
                )
                ptr_pool = c1.enter_context(
                    tc.tile_pool(name="ptrp", bufs=2, space="PSUM")
                )

                nc.vector.memset(v_sb[:, :, 64:65], 1.0)

                prev = None  # (rot, tt) awaiting PE transpose + drain

                def flush_prev():
                    nonlocal prev
                    if prev is None:
                        return
                    rot_p, tp = prev
                    pt = ptr_pool.tile([64, 5, 128], f16, tag="pt")
                    for hh in range(5):
                        nc.tensor.transpose(pt[:, hh, :], rot_p[:, hh, :], id16_sb[:])
                    nc.scalar.copy(qk_sb[:, :, tp * 128 : (tp + 1) * 128], pt[:])
                    prev = None

                xt = None
                for tt in range(NT):
                    if tt % 2 == 0:
                        xt = xt_pool.tile([128, 2, 8, 128], f16, tag="xt")
                        nc.sync.dma_start(xt[:], xt_d[tt // 2])
                        if tt == 0:
                            nc.sync.dma_start(wo_sb[:], wo_d[:])
                            nc.sync.dma_start(md_sb[:], md_d[:])
                            nc.sync.dma_start(mo_sb[:], mo_d[:])
                    pp = pp_pool.tile([128, 6, 64], f32, tag="pp")
                    for kt in range(8):
                        nc.tensor.matmul(
                            pp[:],
                            lhsT=xt[:, tt % 2, kt, :],
                            rhs=wq_sb[:, kt, :],
                            start=(kt == 0),
                            stop=(kt == 7),
                        )
                    # PE: transposes of previous tile (its RoPE is done)
                    flush_prev()
                    # DVE/Pool: RoPE of this tile
                    a = pp[:, 0:5, 0:HALF]
                    b = pp[:, 0:5, HALF:D_K]
                    co = cos_sb[:, tt, :, :].broadcast_to((128, 5, HALF))
                    si = sin_sb[:, tt, :, :].broadcast_to((128, 5, HALF))
                    rot = rot_pool.tile([128, 5, 64], f16, tag="rot")
                    t1 = tmp_pool.tile([128, 5, HALF], f32, tag="t1")
                    t2 = tmp_pool.tile([128, 5, HALF], f32, tag="t2")
                    nc.vector.tensor_tensor(t1[:], a, co, mult)
                    nc.vector.tensor_tensor(t2[:], b, si, mult)
                    nc.gpsimd.tensor_tensor(rot[:, :, 0:HALF], t1[:], t2[:], subtract)
                    t3 = tmp_pool.tile([128, 5, HALF], f32, tag="t1")
                    t4 = tmp_pool.tile([128, 5, HALF], f32, tag="t2")
                    nc.vector.tensor_tensor(t3[:], a, si, mult)
                    nc.vector.tensor_tensor(t4[:], b, co, mult)
                    nc.gpsimd.tensor_tensor(rot[:, :, HALF:D_K], t3[:], t4[:], add)
                    nc.vector.tensor_copy(v_sb[:, tt, 0:64], pp[:, 5, :])
                    prev = (rot, tt)
                flush_prev()

            # ---------------- phase 2: attention + WO partial projection
            with ExitStack() as c2:
                sc_pool = c2.enter_context(
                    tc.tile_pool(name="scp", bufs=2, space="PSUM")
                )
                mix_pool = c2.enter_context(
                    tc.tile_pool(name="mixp", bufs=2, space="PSUM")
                )
                pr_pool = c2.enter_context(tc.tile_pool(name="prp", bufs=4))
                pre_pool = c2.enter_context(tc.tile_pool(name="prep", bufs=8))
                attn_pool = c2.enter_context(tc.tile_pool(name="attnp", bufs=2))
                at_pool = c2.enter_context(tc.tile_pool(name="atp", bufs=2))
                rc_pool = c2.enter_context(tc.tile_pool(name="rcp", bufs=2))
                ob_pool = c2.enter_context(tc.tile_pool(name="obp", bufs=2))

                def issue_scores(i):
                    kb0 = max(0, i - 4)
                    nkb = i - kb0 + 1
                    edge_old = i >= 4
                    scs = []
                    for hp in range(2):
                        scs.append(
                            sc_pool.tile([128, 5, 256], f32, tag="sc", name="sc")
                        )
                    for j in range(nkb):
                        kb = kb0 + j
                        for hp in range(2):
                            nc.tensor.matmul(
                                scs[hp][:, j, :],
                                lhsT=qk_sb[:, 4, kb * 128 : (kb + 1) * 128],
                                rhs=qk_sb[
                                    :, hp * 2 : hp * 2 + 2, i * 128 : (i + 1) * 128
                                ],
                                start=True,
                                stop=True,
                            )
                    prs = []
                    pre_d = []
                    pre_o = []
                    for hp in range(2):
                        pr = pr_pool.tile([128, 5, 256], f16, tag="pr")
                        nc.scalar.activation(
                            pr[:, 0:nkb, :], scs[hp][:, 0:nkb, :], Exp, scale=0.125
                        )
                        ed = pre_pool.tile([128, 256], f16, tag="ed")
                        nc.vector.tensor_tensor(ed[:], pr[:, nkb - 1, :], md_sb[:], mult)
                        eo = None
                        if edge_old:
                            eo = pre_pool.tile([128, 256], f16, tag="eo")
                            nc.vector.tensor_tensor(eo[:], pr[:, 0, :], mo_sb[:], mult)
                        prs.append(pr)
                        pre_d.append(ed)
                        pre_o.append(eo)
                    return (i, kb0, nkb, edge_old, prs, pre_d, pre_o)

                def issue_tail(st):
                    i, kb0, nkb, edge_old, prs, pre_d, pre_o = st
                    av = mix_pool.tile([128, 4, 65], f32, tag="m")
                    unmasked = [
                        j for j in range(nkb - 1) if not (j == 0 and edge_old)
                    ]
                    masked = ([0] if edge_old else []) + [nkb - 1]
                    order = unmasked + masked
                    for h in range(4):
                        hp, hq = h // 2, h % 2
                        for pos, j in enumerate(order):
                            kb = kb0 + j
                            if j == nkb - 1:
                                lhs = pre_d[hp][:, hq * 128 : (hq + 1) * 128]
                            elif j == 0 and edge_old:
                                lhs = pre_o[hp][:, hq * 128 : (hq + 1) * 128]
                            else:
                                lhs = prs[hp][:, j, hq * 128 : (hq + 1) * 128]
                            nc.tensor.matmul(
                                av[:, h, :],
                                lhsT=lhs,
                                rhs=v_sb[:, kb, :],
                                start=(pos == 0),
                                stop=(pos == len(order) - 1),
                            )
                    rc = rc_pool.tile([128, 4, 1], f32, tag="rc")
                    nc.vector.reciprocal(rc[:], av[:, :, 64:65])
                    attn = attn_pool.tile([128, 4, 64], f16, tag="attn")
                    nc.vector.tensor_tensor(
                        attn[:],
                        av[:, :, 0:64],
                        rc[:, :, 0:1].broadcast_to((128, 4, 64)),
                        mult,
                    )
                    atp = mix_pool.tile([128, 2, 128], f16, tag="m")
                    for xx in range(2):
                        nc.tensor.transpose(
                            atp[:, xx, :],
                            attn[:, xx * 2 : (xx + 1) * 2, :],
                            id16_sb[:],
                        )
                    at = at_pool.tile([128, 2, 128], f16, tag="at")
                    nc.vector.tensor_copy(at[:], atp[:])
                    ob = ob_pool.tile([128, 1024], f16, tag="ob")
                    for nb in range(2):
                        po = mix_pool.tile([128, 512], f32, tag="m")
                        for kb2 in range(2):
                            nc.tensor.matmul(
                                po[:],
                                lhsT=at[:, kb2, :],
                                rhs=wo_sb[:, kb2, nb * 512 : (nb + 1) * 512],
                                start=(kb2 == 0),
                                stop=(kb2 == 1),
                            )
                        nc.vector.tensor_copy(ob[:, nb * 512 : (nb + 1) * 512], po[:])
                    nc.sync.dma_start(out_d[i * 128 : (i + 1) * 128, :], ob[:])

                pend = None
                for i in range(NT):
                    st = issue_scores(i)
                    if pend is not None:
                        issue_tail(pend)
                    pend = st
                issue_tail(pend)

    nc.compile()
    return nc


def _host_inputs(x, WQ, WK, WV, WO, token_positions):
    perm64 = np.concatenate([np.arange(0, 64, 2), np.arange(1, 64, 2)])
    pos = np.asarray(token_positions).astype(np.float64)
    inv_freq = THETA ** (-np.arange(HALF, dtype=np.float64) / HALF)
    ang = pos[:, None] * inv_freq[None, :]
    cosr = np.cos(ang).astype(np.float32)
    sinr = np.sin(ang).astype(np.float32)

    def _rope_tiles(r):
        # [T, HALF] -> [128, NT, 1, HALF]
        c = r.reshape(NT, 128, HALF).transpose(1, 0, 2)
        return np.ascontiguousarray(c[:, :, None, :])

    cosb = _rope_tiles(cosr)
    sinb = _rope_tiles(sinr)

    rk = np.arange(128)[:, None]
    r = np.arange(128)[None, :]
    maskd = np.tile((rk <= r).astype(np.float32), (1, 2)).astype(np.float16)
    masko = np.tile((rk >= r).astype(np.float32), (1, 2)).astype(np.float16)
    ident16 = np.eye(128).astype(np.float16)

    in_maps = []
    for core in range(8):
        bi, g = core // 4, core % 4
        WQp = (
            WQ[g * 256 : (g + 1) * 256]
            .reshape(GROUP_SIZE, D_K, D_MODEL)[:, perm64, :]
            .reshape(256, D_MODEL)
        )
        WKp = WK[g * 64 : (g + 1) * 64][perm64, :]
        Wf = np.concatenate([WQp, WKp, WV[g * 64 : (g + 1) * 64]], axis=0)
        wqkvT = np.ascontiguousarray(Wf.T.reshape(8, 128, 384).transpose(1, 0, 2)).astype(
            np.float16
        )
        woT = np.ascontiguousarray(
            WO[:, g * 256 : (g + 1) * 256].T.reshape(2, 128, 1024).transpose(1, 0, 2)
        ).astype(np.float16)
        xT = np.ascontiguousarray(x[bi].T)
        # [1024, T] -> [NT, 128, 8, 128] -> paired tiles [NT/2, 128, 2, 8, 128]
        xt4 = (
            xT.reshape(8, 128, NT, 128)
            .transpose(2, 1, 0, 3)
            .reshape(NT // 2, 2, 128, 8, 128)
            .transpose(0, 2, 1, 3, 4)
        )
        xt4 = np.ascontiguousarray(xt4).astype(np.float16)
        in_maps.append(
            {
                "xt": xt4,
                "wqkvT": wqkvT,
                "woT": woT,
                "cosb": cosb,
                "sinb": sinb,
                "maskd": maskd,
                "masko": masko,
                "ident16": ident16,
            }
        )
    return in_maps


def kernel(x, WQ, WK, WV, WO, token_positions):
    global _PROGRAM
    from concourse.bass_utils import run_bass_kernel_spmd

    x = np.asarray(x, dtype=np.float32)
    WQ = np.asarray(WQ, dtype=np.float32)
    WK = np.asarray(WK, dtype=np.float32)
    WV = np.asarray(WV, dtype=np.float32)
    WO = np.asarray(WO, dtype=np.float32)

    if _PROGRAM is None:
        _PROGRAM = _build_program()
    nc = _PROGRAM

    in_maps = _host_inputs(x, WQ, WK, WV, WO, token_positions)
    res = run_bass_kernel_spmd(nc, in_maps, core_ids=list(range(8)))
    out = np.zeros((B, T, D_MODEL), dtype=np.float32)
    for core in range(8):
        out[core // 4] += res.results[core]["out"].astype(np.float32)
    return out
